# revision 16
# baseline (speedup 1.0000x reference)
"""Trainium2 Bass kernel for nn_GTN_72679436583060 (GTN message passing).

Math: with w-softmax over a singleton axis each GTConv is exactly 2*A, so

    out = 2 * rownorm(4*A@A + I) @ A
        = diag(8 / (4*d + 1)) @ (M@A + 0.25*A)   with M = A@A, d = rowsum(M)

The 0.25*A term is ~2.4e-7 of M@A in relative magnitude (M@A entries are
~5e5, A entries < 1) and is dropped.

Sharding: row-wise over 8 cores, A replicated. Per core (rows R = 256):
  GEMM1 (transposed):  MT = A^T @ (A_rows^T)    (2048 x 256), lhsT = A tiles
  requant:             MT8 = MT/64 cast fp8     (scalar/vector/gpsimd copies)
  GEMM2:               P' = (M/64) @ A          (256 x 2048), lhsT = MT8 tiles
  deg:                 d/64 = rowsum(M/64) via a ones-column matmul
  epilogue:            out = P' / (d/128 + 1/512)  per-row scale, bf16 out

All matmuls run fp8e4 DoubleRow (2 k-subtiles per instruction; measured on
HW this is 1 PE cycle per output row, i.e. 2x bf16 FLOP rate - the PE floor
for the two GEMMs is 65536 cycles/core).  Schedule notes, from traces:
  - The PE clock p-state needs ~3us of continuous work to reach full speed,
    so a run of warm-up matmuls on constant data fills the initial DMA
    window (the first real matmul otherwise runs the whole GEMM1 at half
    clock).
  - t=0's A tile is DMA'd in two column chunks so GEMM1 can start early.
  - GEMM2 runs n-outer so each PSUM bank completes after its own j2 sweep
    and the epilogue + output DMA pipeline behind the next bank's matmuls.
M/64 ~ 8 << 240 = fp8e4 max, A in [0,1); host-validated end-to-end rel err
of this scheme is ~1.6e-3 (gate 2e-2).
"""

import numpy as np

N = 2048
P = 128
NCORES = 8
R = N // NCORES        # 256 rows per core
KP = N // (2 * P)      # 8 k-pair tiles (256 rows each)
FD = 512               # PSUM bank free dim (fp32)
NT2 = N // FD          # 4 GEMM2 n-tiles
WARMUP = 11            # p-state warm-up matmuls (N=256 each)

_CACHE = {}


def _build_bass():
    from contextlib import ExitStack

    import concourse.bass as bass  # noqa: F401
    import concourse.mybir as mybir
    import concourse.tile as tile
    from concourse import bacc

    dt = mybir.dt
    fp32 = dt.float32
    bf16 = dt.bfloat16
    fp8 = dt.float8e4
    Alu = mybir.AluOpType
    Act = mybir.ActivationFunctionType
    DR = mybir.MatmulPerfMode.DoubleRow

    nc = bacc.Bacc(None, target_bir_lowering=False)
    # a_il[t, p, i, c]  = A[t*256 + i*128 + p, c]
    a_d = nc.dram_tensor("a", [KP, P, 2, N], fp8, kind="ExternalInput")
    # art_il[t, p, i, r] = A[row0 + r, t*256 + i*128 + p]
    art_d = nc.dram_tensor("art", [KP, P, 2, R], fp8, kind="ExternalInput")
    out_d = nc.dram_tensor("out", [R, N], bf16, kind="ExternalOutput")

    with tile.TileContext(nc) as tc, ExitStack() as ctx:
        a_pool = ctx.enter_context(tc.tile_pool(name="a", bufs=KP))
        art_pool = ctx.enter_context(tc.tile_pool(name="art", bufs=KP))
        mt_pool = ctx.enter_context(tc.tile_pool(name="mt", bufs=KP))
        const_pool = ctx.enter_context(tc.tile_pool(name="const", bufs=1))
        outsb_pool = ctx.enter_context(tc.tile_pool(name="outsb", bufs=4))
        sc_pool = ctx.enter_context(tc.tile_pool(name="sc", bufs=4))

        warm_t = const_pool.tile([P, 2, R], fp8, tag="warm")
        nc.vector.memset(warm_t[:], 1.0)
        ones_t = const_pool.tile([P, 2, 1], fp8, tag="ones")
        nc.vector.memset(ones_t[:], 1.0)

        # Stream the k-pair tiles; (art[t], a[t]) pairs alternate between
        # the two HWDGE queues (sync/scalar).  Whole-tile transfers keep
        # 4KB-per-partition descriptor rows (splitting every tile measured
        # slower: 24 serialized ~700ns triggers starve the queues); only
        # t=0's A tile is split so GEMM1 can start early.  The per-core DMA
        # ceiling is ~358 GB/s aggregate, which makes the 4.7MB input
        # stream the pacing item for the whole GEMM1 phase.
        a_tiles = [a_pool.tile([P, 2, N], fp8, tag="a", name=f"a_{t}")
                   for t in range(KP)]
        art_tiles = [art_pool.tile([P, 2, R], fp8, tag="art",
                                   name=f"art_{t}") for t in range(KP)]
        # t=0 leads BOTH queues (halves) so nothing competes with its
        # arrival; later tiles alternate whole.
        H = N // 2
        nc.sync.dma_start(art_tiles[0][:], art_d[0])
        nc.sync.dma_start(a_tiles[0][:, :, 0:H], a_d[0][:, :, 0:H])
        nc.scalar.dma_start(a_tiles[0][:, :, H:N], a_d[0][:, :, H:N])
        for t in range(1, KP):
            eng = nc.scalar if t % 2 == 1 else nc.sync
            eng.dma_start(art_tiles[t][:], art_d[t])
            eng.dma_start(a_tiles[t][:], a_d[t])

        # ---- GEMM1: MT[j*128+m, r] = sum_k A[k, j*128+m] * A[row0+r, k] ----
        # DoubleRow, t-outer so the PE tracks the streaming A DMA.  Each
        # PSUM bank holds one j-pair (two [128, 256] MT tiles = the exact
        # DoubleRow k-pair layout GEMM2's lhsT wants).
        with tc.tile_pool(name="psum", bufs=8, space="PSUM") as psum_pool:
            pairs = [psum_pool.tile([P, 2, R], fp32, tag="bank",
                                    name=f"pair_{b}") for b in range(KP)]
            # Warm-up: garbage matmuls on the const tile raise the PE
            # p-state during the DMA window.  They write pairs[7], whose
            # first real matmul below has start=True and so re-marks the
            # whole bank pending-zero (the PE runs its queue in order).
            for w in range(WARMUP):
                nc.tensor.matmul(
                    pairs[KP - 1][:, w % 2, :], warm_t[:, :, 0:P],
                    warm_t[:], start=(w == 0), stop=False,
                    perf_mode=DR, skip_group_check=True,
                )
            # Bank init rides on the t=0 matmuls: the half-0 matmul has
            # start=True -> marks the whole bank pending-zero; the half-1
            # matmul (start=False, program-ordered after it) writes into
            # still-pending bytes and therefore also overwrites.
            for t in range(KP):
                for j2 in range(KP):
                    for half in range(2):
                        j = 2 * j2 + half
                        nc.tensor.matmul(
                            pairs[j2][:, half, :],
                            a_tiles[t][:, :, j * P:(j + 1) * P],
                            art_tiles[t][:],
                            start=(t == 0 and half == 0),
                            stop=(t == KP - 1),
                            perf_mode=DR, skip_group_check=True,
                        )

            # Requantize MT -> fp8 (MT/64), alternating the scalar and
            # vector engines so two copies drain per GEMM2 j2-round.
            # (GPSIMD cannot access PSUM.)
            mt_tiles = []
            for j2 in range(KP):
                mt = mt_pool.tile([P, 2, R], fp8, tag="mt")
                if j2 % 2 == 0:
                    nc.scalar.activation(mt[:], pairs[j2][:], Act.Copy,
                                         scale=1.0 / 64.0)
                else:
                    nc.vector.tensor_scalar(
                        out=mt[:], in0=pairs[j2][:], scalar1=1.0 / 64.0,
                        scalar2=None, op0=Alu.mult,
                    )
                mt_tiles.append(mt)

            # ---- GEMM2 + deg + epilogue, n-outer ----
            def emit_deg_scale(m, deg_ps):
                # psum deg = d/64;  scale = 1 / (d/128 + 1/512)
                t1 = sc_pool.tile([P, 1], fp32, tag="t1", name=f"t1_{m}")
                nc.vector.tensor_scalar(
                    out=t1[:], in0=deg_ps[:], scalar1=0.5,
                    scalar2=1.0 / 512.0, op0=Alu.mult, op1=Alu.add,
                )
                sca = sc_pool.tile([P, 1], fp32, tag="sca", name=f"sca_{m}")
                nc.vector.reciprocal(sca[:], t1[:])
                return sca

            def emit_epilogue(m, n, psum_tile, sca, split=False):
                ot = outsb_pool.tile([P, FD], bf16, tag="ot",
                                     name=f"ot_{m}_{n}")
                if not split:
                    nc.vector.tensor_scalar(
                        out=ot[:], in0=psum_tile[:], scalar1=sca[:],
                        scalar2=None, op0=Alu.mult,
                    )
                    eng = nc.sync if n % 2 == 0 else nc.scalar
                    eng.dma_start(
                        out_d[m * P:(m + 1) * P, n * FD:(n + 1) * FD], ot[:]
                    )
                    return
                # Final bank: halve the scale + store across both compute
                # engines and both DMA queues to shorten the serial tail.
                hf = FD // 2
                nc.vector.tensor_scalar(
                    out=ot[:, 0:hf], in0=psum_tile[:, 0:hf], scalar1=sca[:],
                    scalar2=None, op0=Alu.mult,
                )
                nc.scalar.activation(ot[:, hf:FD], psum_tile[:, hf:FD],
                                     Act.Copy, scale=sca[:])
                nc.sync.dma_start(
                    out_d[m * P:(m + 1) * P,
                          n * FD:n * FD + hf], ot[:, 0:hf]
                )
                nc.scalar.dma_start(
                    out_d[m * P:(m + 1) * P,
                          n * FD + hf:(n + 1) * FD], ot[:, hf:FD]
                )

            for m in range(2):
                deg_full = None
                deg_ps = None
                sca = None
                for n in range(NT2):
                    ops = psum_pool.tile([P, FD], fp32, tag="bank",
                                         name=f"outps{m}_{n}")
                    if n == 0:
                        deg_full = psum_pool.tile([P, FD], fp32, tag="bank",
                                                  name=f"deg_{m}")
                        deg_ps = deg_full[:, 0:1]
                    last = (m == 1 and n == NT2 - 1)
                    if not last:
                        for j2 in range(KP):
                            lhsT = mt_tiles[j2][:, :, m * P:(m + 1) * P]
                            nc.tensor.matmul(
                                ops[:], lhsT,
                                a_tiles[j2][:, :, n * FD:(n + 1) * FD],
                                start=(j2 == 0), stop=(j2 == KP - 1),
                                perf_mode=DR,
                            )
                            if n == 0:
                                nc.tensor.matmul(
                                    deg_ps[:], lhsT, ones_t[:],
                                    start=(j2 == 0), stop=(j2 == KP - 1),
                                    perf_mode=DR,
                                )
                        if n == 0:
                            sca = emit_deg_scale(m, deg_ps)
                        emit_epilogue(m, n, ops, sca)
                        continue
                    # Final bank: run the two 256-wide halves as separate
                    # sweeps so each half's epilogue + store overlaps the
                    # other half / the teardown fires sooner.  Half-0's
                    # start=True marks the whole bank pending-zero, so
                    # half-1's first (start=False, program-ordered later)
                    # write still overwrites.
                    hf = FD // 2
                    ot = outsb_pool.tile([P, FD], bf16, tag="ot",
                                         name="ot_last")
                    for half in range(2):
                        lo = n * FD + half * hf
                        for j2 in range(KP):
                            lhsT = mt_tiles[j2][:, :, m * P:(m + 1) * P]
                            nc.tensor.matmul(
                                ops[:, half * hf:(half + 1) * hf], lhsT,
                                a_tiles[j2][:, :, lo:lo + hf],
                                start=(half == 0 and j2 == 0),
                                stop=(j2 == KP - 1),
                                perf_mode=DR, skip_group_check=True,
                            )
                        src = ops[:, half * hf:(half + 1) * hf]
                        dst = ot[:, half * hf:(half + 1) * hf]
                        if half == 0:
                            nc.vector.tensor_scalar(
                                out=dst, in0=src, scalar1=sca[:],
                                scalar2=None, op0=Alu.mult,
                            )
                            nc.sync.dma_start(
                                out_d[m * P:(m + 1) * P, lo:lo + hf], dst)
                        else:
                            nc.scalar.activation(dst, src, Act.Copy,
                                                 scale=sca[:])
                            nc.scalar.dma_start(
                                out_d[m * P:(m + 1) * P, lo:lo + hf], dst)
    nc.compile()
    return nc


def _get_nc():
    if "nc" not in _CACHE:
        _CACHE["nc"] = _build_bass()
    return _CACHE["nc"]


def _make_in_maps(A_f32):
    import ml_dtypes

    f8 = ml_dtypes.float8_e4m3
    A8 = A_f32.astype(f8)
    # a_il[t, p, i, c] = A[t*256 + i*128 + p, c]
    a_il = np.ascontiguousarray(
        A8.reshape(KP, 2, P, N).transpose(0, 2, 1, 3)
    )
    AT8 = A8.T
    in_maps = []
    for c in range(NCORES):
        sl = slice(c * R, (c + 1) * R)
        # art_il[t, p, i, r] = A[row0 + r, t*256 + i*128 + p]
        art_il = np.ascontiguousarray(
            AT8[:, sl].reshape(KP, 2, P, R).transpose(0, 2, 1, 3)
        )
        in_maps.append({"a": a_il, "art": art_il})
    return in_maps


def kernel(A, w1a=None, w1b=None, w2a=None, **_unused):
    # w1a/w1b/w2a only enter the reference through a softmax over a
    # singleton axis (== 1.0), so the output does not depend on them.
    from concourse.bass_utils import run_bass_kernel_spmd

    A = np.asarray(A, dtype=np.float32)
    assert A.shape == (N, N), A.shape
    nc = _get_nc()
    in_maps = _make_in_maps(A)
    res = run_bass_kernel_spmd(nc, in_maps, core_ids=list(range(NCORES)))
    out = np.concatenate(
        [res.results[c]["out"] for c in range(NCORES)], axis=0
    )
    return out[None].astype(np.float32)


# revision 18
# speedup vs baseline: 1.0352x; 1.0352x over previous
"""Trainium2 Bass kernel for nn_GTN_72679436583060 (GTN message passing).

Math: with w-softmax over a singleton axis each GTConv is exactly 2*A, so

    out = 2 * rownorm(4*A@A + I) @ A
        = diag(8 / (4*d + 1)) @ (M@A + 0.25*A)   with M = A@A, d = rowsum(M)

The 0.25*A term is ~2.4e-7 of M@A in relative magnitude (M@A entries are
~5e5, A entries < 1) and is dropped.

Sharding: row-wise over 8 cores, A replicated. Per core (rows R = 256):
  GEMM1 (transposed):  MT = A^T @ (A_rows^T)    (2048 x 256), lhsT = A tiles
  requant:             MT8 = MT/64 cast fp8     (scalar/vector/gpsimd copies)
  GEMM2:               P' = (M/64) @ A          (256 x 2048), lhsT = MT8 tiles
  deg:                 d/64 = rowsum(M/64) via a ones-column matmul
  epilogue:            out = P' / (d/128 + 1/512)  per-row scale, bf16 out

All matmuls run fp8e4 DoubleRow (2 k-subtiles per instruction; measured on
HW this is 1 PE cycle per output row, i.e. 2x bf16 FLOP rate - the PE floor
for the two GEMMs is 65536 cycles/core).  Schedule notes, from traces:
  - The PE clock p-state needs ~3us of continuous work to reach full speed,
    so a run of warm-up matmuls on constant data fills the initial DMA
    window (the first real matmul otherwise runs the whole GEMM1 at half
    clock).
  - t=0's A tile is DMA'd in two column chunks so GEMM1 can start early.
  - GEMM2 runs n-outer so each PSUM bank completes after its own j2 sweep
    and the epilogue + output DMA pipeline behind the next bank's matmuls.
M/64 ~ 8 << 240 = fp8e4 max, A in [0,1); host-validated end-to-end rel err
of this scheme is ~1.6e-3 (gate 2e-2).
"""

import numpy as np

N = 2048
P = 128
NCORES = 8
R = N // NCORES        # 256 rows per core
KP = N // (2 * P)      # 8 k-pair tiles (256 rows each)
FD = 512               # PSUM bank free dim (fp32)
NT2 = N // FD          # 4 GEMM2 n-tiles
WARMUP = 12            # p-state warm-up matmuls (N=256 each)

_CACHE = {}


def _build_bass():
    from contextlib import ExitStack

    import concourse.bass as bass  # noqa: F401
    import concourse.mybir as mybir
    import concourse.tile as tile
    from concourse import bacc

    dt = mybir.dt
    fp32 = dt.float32
    bf16 = dt.bfloat16
    fp8 = dt.float8e4
    Alu = mybir.AluOpType
    Act = mybir.ActivationFunctionType
    DR = mybir.MatmulPerfMode.DoubleRow

    nc = bacc.Bacc(None, target_bir_lowering=False)
    # a_il[t, p, i, c]  = A[t*256 + i*128 + p, c]
    a_d = nc.dram_tensor("a", [KP, P, 2, N], fp8, kind="ExternalInput")
    # art_il[t, p, i, r] = A[row0 + r, t*256 + i*128 + p]
    art_d = nc.dram_tensor("art", [KP, P, 2, R], fp8, kind="ExternalInput")
    out_d = nc.dram_tensor("out", [R, N], bf16, kind="ExternalOutput")

    with tile.TileContext(nc) as tc, ExitStack() as ctx:
        a_pool = ctx.enter_context(tc.tile_pool(name="a", bufs=KP))
        art_pool = ctx.enter_context(tc.tile_pool(name="art", bufs=KP))
        mt_pool = ctx.enter_context(tc.tile_pool(name="mt", bufs=KP))
        const_pool = ctx.enter_context(tc.tile_pool(name="const", bufs=1))
        outsb_pool = ctx.enter_context(tc.tile_pool(name="outsb", bufs=4))
        sc_pool = ctx.enter_context(tc.tile_pool(name="sc", bufs=4))

        warm_t = const_pool.tile([P, 2, R], fp8, tag="warm")
        nc.vector.memset(warm_t[:], 1.0)
        ones_t = const_pool.tile([P, 2, 1], fp8, tag="ones")
        nc.vector.memset(ones_t[:], 1.0)

        # Stream the k-pair tiles; (art[t], a[t]) pairs alternate between
        # the two HWDGE queues (sync/scalar).  Whole-tile transfers keep
        # 4KB-per-partition descriptor rows (splitting every tile measured
        # slower: 24 serialized ~700ns triggers starve the queues); only
        # t=0's A tile is split so GEMM1 can start early.  The per-core DMA
        # ceiling is ~358 GB/s aggregate, which makes the 4.7MB input
        # stream the pacing item for the whole GEMM1 phase.
        a_tiles = [a_pool.tile([P, 2, N], fp8, tag="a", name=f"a_{t}")
                   for t in range(KP)]
        art_tiles = [art_pool.tile([P, 2, R], fp8, tag="art",
                                   name=f"art_{t}") for t in range(KP)]
        # t=0 leads BOTH queues (halves) so nothing competes with its
        # arrival; later tiles alternate whole, and t=7 is split the same
        # way so both queues carry exactly 2.3MB and finish together.
        H = N // 2
        nc.sync.dma_start(art_tiles[0][:], art_d[0])
        nc.sync.dma_start(a_tiles[0][:, :, 0:H], a_d[0][:, :, 0:H])
        nc.scalar.dma_start(a_tiles[0][:, :, H:N], a_d[0][:, :, H:N])
        for t in range(1, KP - 1):
            eng = nc.scalar if t % 2 == 1 else nc.sync
            eng.dma_start(art_tiles[t][:], art_d[t])
            eng.dma_start(a_tiles[t][:], a_d[t])
        t = KP - 1
        nc.scalar.dma_start(art_tiles[t][:], art_d[t])
        nc.sync.dma_start(a_tiles[t][:, :, 0:H], a_d[t][:, :, 0:H])
        nc.scalar.dma_start(a_tiles[t][:, :, H:N], a_d[t][:, :, H:N])

        # ---- GEMM1: MT[j*128+m, r] = sum_k A[k, j*128+m] * A[row0+r, k] ----
        # DoubleRow, t-outer so the PE tracks the streaming A DMA.  Each
        # PSUM bank holds one j-pair (two [128, 256] MT tiles = the exact
        # DoubleRow k-pair layout GEMM2's lhsT wants).
        with tc.tile_pool(name="psum", bufs=8, space="PSUM") as psum_pool:
            pairs = [psum_pool.tile([P, 2, R], fp32, tag="bank",
                                    name=f"pair_{b}") for b in range(KP)]
            # Warm-up: garbage matmuls on the const tile raise the PE
            # p-state during the DMA window.  They write pairs[7], whose
            # first real matmul below has start=True and so re-marks the
            # whole bank pending-zero (the PE runs its queue in order).
            for w in range(WARMUP):
                nc.tensor.matmul(
                    pairs[KP - 1][:, w % 2, :], warm_t[:, :, 0:P],
                    warm_t[:], start=(w == 0), stop=False,
                    perf_mode=DR, skip_group_check=True,
                )
            # Bank init rides on the t=0 matmuls: the half-0 matmul has
            # start=True -> marks the whole bank pending-zero; the half-1
            # matmul (start=False, program-ordered after it) writes into
            # still-pending bytes and therefore also overwrites.
            for t in range(KP):
                for j2 in range(KP):
                    for half in range(2):
                        j = 2 * j2 + half
                        nc.tensor.matmul(
                            pairs[j2][:, half, :],
                            a_tiles[t][:, :, j * P:(j + 1) * P],
                            art_tiles[t][:],
                            start=(t == 0 and half == 0),
                            stop=(t == KP - 1),
                            perf_mode=DR, skip_group_check=True,
                        )

            # Requantize MT -> fp8 (MT/64), alternating the scalar and
            # vector engines so two copies drain per GEMM2 j2-round.
            # (GPSIMD cannot access PSUM.)
            mt_tiles = []
            for j2 in range(KP):
                mt = mt_pool.tile([P, 2, R], fp8, tag="mt")
                if j2 % 2 == 0:
                    nc.scalar.activation(mt[:], pairs[j2][:], Act.Copy,
                                         scale=1.0 / 64.0)
                else:
                    nc.vector.tensor_scalar(
                        out=mt[:], in0=pairs[j2][:], scalar1=1.0 / 64.0,
                        scalar2=None, op0=Alu.mult,
                    )
                mt_tiles.append(mt)

            # ---- GEMM2 + deg + epilogue, n-outer ----
            def emit_deg_scale(m, deg_ps):
                # psum deg = d/64;  scale = 1 / (d/128 + 1/512)
                t1 = sc_pool.tile([P, 1], fp32, tag="t1", name=f"t1_{m}")
                nc.vector.tensor_scalar(
                    out=t1[:], in0=deg_ps[:], scalar1=0.5,
                    scalar2=1.0 / 512.0, op0=Alu.mult, op1=Alu.add,
                )
                sca = sc_pool.tile([P, 1], fp32, tag="sca", name=f"sca_{m}")
                nc.vector.reciprocal(sca[:], t1[:])
                return sca

            def emit_epilogue(m, n, psum_tile, sca, split=False):
                ot = outsb_pool.tile([P, FD], bf16, tag="ot",
                                     name=f"ot_{m}_{n}")
                if not split:
                    nc.vector.tensor_scalar(
                        out=ot[:], in0=psum_tile[:], scalar1=sca[:],
                        scalar2=None, op0=Alu.mult,
                    )
                    eng = nc.sync if n % 2 == 0 else nc.scalar
                    eng.dma_start(
                        out_d[m * P:(m + 1) * P, n * FD:(n + 1) * FD], ot[:]
                    )
                    return
                # Final bank: halve the scale + store across both compute
                # engines and both DMA queues to shorten the serial tail.
                hf = FD // 2
                nc.vector.tensor_scalar(
                    out=ot[:, 0:hf], in0=psum_tile[:, 0:hf], scalar1=sca[:],
                    scalar2=None, op0=Alu.mult,
                )
                nc.scalar.activation(ot[:, hf:FD], psum_tile[:, hf:FD],
                                     Act.Copy, scale=sca[:])
                nc.sync.dma_start(
                    out_d[m * P:(m + 1) * P,
                          n * FD:n * FD + hf], ot[:, 0:hf]
                )
                nc.scalar.dma_start(
                    out_d[m * P:(m + 1) * P,
                          n * FD + hf:(n + 1) * FD], ot[:, hf:FD]
                )

            for m in range(2):
                deg_full = None
                deg_ps = None
                sca = None
                for n in range(NT2):
                    ops = psum_pool.tile([P, FD], fp32, tag="bank",
                                         name=f"outps{m}_{n}")
                    if n == 0:
                        deg_full = psum_pool.tile([P, FD], fp32, tag="bank",
                                                  name=f"deg_{m}")
                        deg_ps = deg_full[:, 0:1]
                    last = (m == 1 and n == NT2 - 1)
                    if not last:
                        for j2 in range(KP):
                            lhsT = mt_tiles[j2][:, :, m * P:(m + 1) * P]
                            nc.tensor.matmul(
                                ops[:], lhsT,
                                a_tiles[j2][:, :, n * FD:(n + 1) * FD],
                                start=(j2 == 0), stop=(j2 == KP - 1),
                                perf_mode=DR,
                            )
                            if n == 0:
                                nc.tensor.matmul(
                                    deg_ps[:], lhsT, ones_t[:],
                                    start=(j2 == 0), stop=(j2 == KP - 1),
                                    perf_mode=DR,
                                )
                        if n == 0:
                            sca = emit_deg_scale(m, deg_ps)
                        emit_epilogue(m, n, ops, sca)
                        continue
                    # Final bank: run the two 256-wide halves as separate
                    # sweeps so each half's epilogue + store overlaps the
                    # other half / the teardown fires sooner.  Half-0's
                    # start=True marks the whole bank pending-zero, so
                    # half-1's first (start=False, program-ordered later)
                    # write still overwrites.
                    hf = FD // 2
                    ot = outsb_pool.tile([P, FD], bf16, tag="ot",
                                         name="ot_last")
                    for half in range(2):
                        lo = n * FD + half * hf
                        for j2 in range(KP):
                            lhsT = mt_tiles[j2][:, :, m * P:(m + 1) * P]
                            nc.tensor.matmul(
                                ops[:, half * hf:(half + 1) * hf], lhsT,
                                a_tiles[j2][:, :, lo:lo + hf],
                                start=(half == 0 and j2 == 0),
                                stop=(j2 == KP - 1),
                                perf_mode=DR, skip_group_check=True,
                            )
                        src = ops[:, half * hf:(half + 1) * hf]
                        dst = ot[:, half * hf:(half + 1) * hf]
                        if half == 0:
                            nc.vector.tensor_scalar(
                                out=dst, in0=src, scalar1=sca[:],
                                scalar2=None, op0=Alu.mult,
                            )
                            nc.sync.dma_start(
                                out_d[m * P:(m + 1) * P, lo:lo + hf], dst)
                        else:
                            nc.scalar.activation(dst, src, Act.Copy,
                                                 scale=sca[:])
                            nc.scalar.dma_start(
                                out_d[m * P:(m + 1) * P, lo:lo + hf], dst)
    nc.compile()
    return nc


def _get_nc():
    if "nc" not in _CACHE:
        _CACHE["nc"] = _build_bass()
    return _CACHE["nc"]


def _make_in_maps(A_f32):
    import ml_dtypes

    f8 = ml_dtypes.float8_e4m3
    A8 = A_f32.astype(f8)
    # a_il[t, p, i, c] = A[t*256 + i*128 + p, c]
    a_il = np.ascontiguousarray(
        A8.reshape(KP, 2, P, N).transpose(0, 2, 1, 3)
    )
    AT8 = A8.T
    in_maps = []
    for c in range(NCORES):
        sl = slice(c * R, (c + 1) * R)
        # art_il[t, p, i, r] = A[row0 + r, t*256 + i*128 + p]
        art_il = np.ascontiguousarray(
            AT8[:, sl].reshape(KP, 2, P, R).transpose(0, 2, 1, 3)
        )
        in_maps.append({"a": a_il, "art": art_il})
    return in_maps


def kernel(A, w1a=None, w1b=None, w2a=None, **_unused):
    # w1a/w1b/w2a only enter the reference through a softmax over a
    # singleton axis (== 1.0), so the output does not depend on them.
    from concourse.bass_utils import run_bass_kernel_spmd

    A = np.asarray(A, dtype=np.float32)
    assert A.shape == (N, N), A.shape
    nc = _get_nc()
    in_maps = _make_in_maps(A)
    res = run_bass_kernel_spmd(nc, in_maps, core_ids=list(range(NCORES)))
    out = np.concatenate(
        [res.results[c]["out"] for c in range(NCORES)], axis=0
    )
    return out[None].astype(np.float32)


# revision 19
# speedup vs baseline: 1.0703x; 1.0339x over previous
"""Trainium2 Bass kernel for nn_GTN_72679436583060 (GTN message passing).

Math: with w-softmax over a singleton axis each GTConv is exactly 2*A, so

    out = 2 * rownorm(4*A@A + I) @ A
        = diag(8 / (4*d + 1)) @ (M@A + 0.25*A)   with M = A@A, d = rowsum(M)

The 0.25*A term is ~2.4e-7 of M@A in relative magnitude (M@A entries are
~5e5, A entries < 1) and is dropped.

Sharding: row-wise over 8 cores, A replicated. Per core (rows R = 256):
  GEMM1 (transposed):  MT = A^T @ (A_rows^T)    (2048 x 256), lhsT = A tiles
  requant:             MT8 = MT/64 cast fp8     (scalar/vector/gpsimd copies)
  GEMM2:               P' = (M/64) @ A          (256 x 2048), lhsT = MT8 tiles
  deg:                 d/64 = rowsum(M/64) via a ones-column matmul
  epilogue:            out = P' / (d/128 + 1/512)  per-row scale, bf16 out

All matmuls run fp8e4 DoubleRow (2 k-subtiles per instruction; measured on
HW this is 1 PE cycle per output row, i.e. 2x bf16 FLOP rate - the PE floor
for the two GEMMs is 65536 cycles/core).  Schedule notes, from traces:
  - The PE clock p-state needs ~3us of continuous work to reach full speed,
    so a run of warm-up matmuls on constant data fills the initial DMA
    window (the first real matmul otherwise runs the whole GEMM1 at half
    clock).
  - t=0's A tile is DMA'd in two column chunks so GEMM1 can start early.
  - GEMM2 runs n-outer so each PSUM bank completes after its own j2 sweep
    and the epilogue + output DMA pipeline behind the next bank's matmuls.
M/64 ~ 8 << 240 = fp8e4 max, A in [0,1); host-validated end-to-end rel err
of this scheme is ~1.6e-3 (gate 2e-2).
"""

import numpy as np

N = 2048
P = 128
NCORES = 8
R = N // NCORES        # 256 rows per core
KP = N // (2 * P)      # 8 k-pair tiles (256 rows each)
FD = 512               # PSUM bank free dim (fp32)
NT2 = N // FD          # 4 GEMM2 n-tiles
WARMUP = 12            # p-state warm-up matmuls (N=256 each)

_CACHE = {}


def _build_bass():
    from contextlib import ExitStack

    import concourse.bass as bass  # noqa: F401
    import concourse.mybir as mybir
    import concourse.tile as tile
    from concourse import bacc

    dt = mybir.dt
    fp32 = dt.float32
    bf16 = dt.bfloat16
    fp8 = dt.float8e4
    Alu = mybir.AluOpType
    Act = mybir.ActivationFunctionType
    DR = mybir.MatmulPerfMode.DoubleRow

    nc = bacc.Bacc(None, target_bir_lowering=False)
    # a_il[t, p, i, c]  = A[t*256 + i*128 + p, c]
    a_d = nc.dram_tensor("a", [KP, P, 2, N], fp8, kind="ExternalInput")
    # art_il[t, p, i, r] = A[row0 + r, t*256 + i*128 + p]
    art_d = nc.dram_tensor("art", [KP, P, 2, R], fp8, kind="ExternalInput")
    out_d = nc.dram_tensor("out", [R, N], bf16, kind="ExternalOutput")

    with tile.TileContext(nc) as tc, ExitStack() as ctx:
        a_pool = ctx.enter_context(tc.tile_pool(name="a", bufs=KP))
        art_pool = ctx.enter_context(tc.tile_pool(name="art", bufs=KP))
        mt_pool = ctx.enter_context(tc.tile_pool(name="mt", bufs=KP))
        const_pool = ctx.enter_context(tc.tile_pool(name="const", bufs=1))
        outsb_pool = ctx.enter_context(tc.tile_pool(name="outsb", bufs=4))
        sc_pool = ctx.enter_context(tc.tile_pool(name="sc", bufs=4))

        warm_t = const_pool.tile([P, 2, R], fp8, tag="warm")
        nc.vector.memset(warm_t[:], 1.0)
        ones_t = const_pool.tile([P, 2, 1], fp8, tag="ones")
        nc.vector.memset(ones_t[:], 1.0)

        # Stream the k-pair tiles; (art[t], a[t]) pairs alternate between
        # the two HWDGE queues (sync/scalar).  Whole-tile transfers keep
        # 4KB-per-partition descriptor rows (splitting every tile measured
        # slower: 24 serialized ~700ns triggers starve the queues); only
        # t=0's A tile is split so GEMM1 can start early.  The per-core DMA
        # ceiling is ~358 GB/s aggregate, which makes the 4.7MB input
        # stream the pacing item for the whole GEMM1 phase.
        a_tiles = [a_pool.tile([P, 2, N], fp8, tag="a", name=f"a_{t}")
                   for t in range(KP)]
        art_tiles = [art_pool.tile([P, 2, R], fp8, tag="art",
                                   name=f"art_{t}") for t in range(KP)]
        # t=0 leads BOTH queues (halves) so nothing competes with its
        # arrival; later tiles alternate whole (2KB+ descriptor rows —
        # further splitting measurably slows the aggregate stream), with
        # art7 moved to sync to trim the queue-byte skew.
        H = N // 2
        nc.sync.dma_start(art_tiles[0][:], art_d[0])
        nc.sync.dma_start(a_tiles[0][:, :, 0:H], a_d[0][:, :, 0:H])
        nc.scalar.dma_start(a_tiles[0][:, :, H:N], a_d[0][:, :, H:N])
        for t in range(1, KP - 1):
            eng = nc.scalar if t % 2 == 1 else nc.sync
            eng.dma_start(art_tiles[t][:], art_d[t])
            eng.dma_start(a_tiles[t][:], a_d[t])
        t = KP - 1
        nc.sync.dma_start(art_tiles[t][:], art_d[t])
        nc.scalar.dma_start(a_tiles[t][:], a_d[t])

        # ---- GEMM1: MT[j*128+m, r] = sum_k A[k, j*128+m] * A[row0+r, k] ----
        # DoubleRow, t-outer so the PE tracks the streaming A DMA.  Each
        # PSUM bank holds one j-pair (two [128, 256] MT tiles = the exact
        # DoubleRow k-pair layout GEMM2's lhsT wants).
        with tc.tile_pool(name="psum", bufs=8, space="PSUM") as psum_pool:
            pairs = [psum_pool.tile([P, 2, R], fp32, tag="bank",
                                    name=f"pair_{b}") for b in range(KP)]
            # Warm-up: garbage matmuls on the const tile raise the PE
            # p-state during the DMA window.  They write pairs[7], whose
            # first real matmul below has start=True and so re-marks the
            # whole bank pending-zero (the PE runs its queue in order).
            for w in range(WARMUP):
                nc.tensor.matmul(
                    pairs[KP - 1][:, w % 2, :], warm_t[:, :, 0:P],
                    warm_t[:], start=(w == 0), stop=False,
                    perf_mode=DR, skip_group_check=True,
                )
            # Bank init rides on the t=0 matmuls: the half-0 matmul has
            # start=True -> marks the whole bank pending-zero; the half-1
            # matmul (start=False, program-ordered after it) writes into
            # still-pending bytes and therefore also overwrites.
            for t in range(KP):
                for j2 in range(KP):
                    for half in range(2):
                        j = 2 * j2 + half
                        nc.tensor.matmul(
                            pairs[j2][:, half, :],
                            a_tiles[t][:, :, j * P:(j + 1) * P],
                            art_tiles[t][:],
                            start=(t == 0 and half == 0),
                            stop=(t == KP - 1),
                            perf_mode=DR, skip_group_check=True,
                        )

            # Requantize MT -> fp8 (MT/64), alternating the scalar and
            # vector engines so two copies drain per GEMM2 j2-round.
            # (GPSIMD cannot access PSUM.)
            mt_tiles = []
            for j2 in range(KP):
                mt = mt_pool.tile([P, 2, R], fp8, tag="mt")
                if j2 % 2 == 0:
                    nc.scalar.activation(mt[:], pairs[j2][:], Act.Copy,
                                         scale=1.0 / 64.0)
                else:
                    nc.vector.tensor_scalar(
                        out=mt[:], in0=pairs[j2][:], scalar1=1.0 / 64.0,
                        scalar2=None, op0=Alu.mult,
                    )
                mt_tiles.append(mt)

            # ---- GEMM2 + deg + epilogue, n-outer ----
            def emit_deg_scale(m, deg_ps):
                # psum deg = d/64;  scale = 1 / (d/128 + 1/512)
                t1 = sc_pool.tile([P, 1], fp32, tag="t1", name=f"t1_{m}")
                nc.vector.tensor_scalar(
                    out=t1[:], in0=deg_ps[:], scalar1=0.5,
                    scalar2=1.0 / 512.0, op0=Alu.mult, op1=Alu.add,
                )
                sca = sc_pool.tile([P, 1], fp32, tag="sca", name=f"sca_{m}")
                nc.vector.reciprocal(sca[:], t1[:])
                return sca

            def emit_epilogue(m, n, psum_tile, sca, split=False):
                ot = outsb_pool.tile([P, FD], bf16, tag="ot",
                                     name=f"ot_{m}_{n}")
                if not split:
                    nc.vector.tensor_scalar(
                        out=ot[:], in0=psum_tile[:], scalar1=sca[:],
                        scalar2=None, op0=Alu.mult,
                    )
                    eng = nc.sync if n % 2 == 0 else nc.scalar
                    eng.dma_start(
                        out_d[m * P:(m + 1) * P, n * FD:(n + 1) * FD], ot[:]
                    )
                    return
                # Final bank: halve the scale + store across both compute
                # engines and both DMA queues to shorten the serial tail.
                hf = FD // 2
                nc.vector.tensor_scalar(
                    out=ot[:, 0:hf], in0=psum_tile[:, 0:hf], scalar1=sca[:],
                    scalar2=None, op0=Alu.mult,
                )
                nc.scalar.activation(ot[:, hf:FD], psum_tile[:, hf:FD],
                                     Act.Copy, scale=sca[:])
                nc.sync.dma_start(
                    out_d[m * P:(m + 1) * P,
                          n * FD:n * FD + hf], ot[:, 0:hf]
                )
                nc.scalar.dma_start(
                    out_d[m * P:(m + 1) * P,
                          n * FD + hf:(n + 1) * FD], ot[:, hf:FD]
                )

            for m in range(2):
                deg_full = None
                deg_ps = None
                sca = None
                for n in range(NT2):
                    ops = psum_pool.tile([P, FD], fp32, tag="bank",
                                         name=f"outps{m}_{n}")
                    if n == 0:
                        deg_full = psum_pool.tile([P, FD], fp32, tag="bank",
                                                  name=f"deg_{m}")
                        deg_ps = deg_full[:, 0:1]
                    last = (m == 1 and n == NT2 - 1)
                    if not last:
                        for j2 in range(KP):
                            lhsT = mt_tiles[j2][:, :, m * P:(m + 1) * P]
                            nc.tensor.matmul(
                                ops[:], lhsT,
                                a_tiles[j2][:, :, n * FD:(n + 1) * FD],
                                start=(j2 == 0), stop=(j2 == KP - 1),
                                perf_mode=DR,
                            )
                            if n == 0:
                                nc.tensor.matmul(
                                    deg_ps[:], lhsT, ones_t[:],
                                    start=(j2 == 0), stop=(j2 == KP - 1),
                                    perf_mode=DR,
                                )
                        if n == 0:
                            sca = emit_deg_scale(m, deg_ps)
                        emit_epilogue(m, n, ops, sca)
                        continue
                    # Final bank: run the two 256-wide halves as separate
                    # sweeps so each half's epilogue + store overlaps the
                    # other half / the teardown fires sooner.  Half-0's
                    # start=True marks the whole bank pending-zero, so
                    # half-1's first (start=False, program-ordered later)
                    # write still overwrites.
                    hf = FD // 2
                    ot = outsb_pool.tile([P, FD], bf16, tag="ot",
                                         name="ot_last")
                    for half in range(2):
                        lo = n * FD + half * hf
                        for j2 in range(KP):
                            lhsT = mt_tiles[j2][:, :, m * P:(m + 1) * P]
                            nc.tensor.matmul(
                                ops[:, half * hf:(half + 1) * hf], lhsT,
                                a_tiles[j2][:, :, lo:lo + hf],
                                start=(half == 0 and j2 == 0),
                                stop=(j2 == KP - 1),
                                perf_mode=DR, skip_group_check=True,
                            )
                        src = ops[:, half * hf:(half + 1) * hf]
                        dst = ot[:, half * hf:(half + 1) * hf]
                        if half == 0:
                            nc.vector.tensor_scalar(
                                out=dst, in0=src, scalar1=sca[:],
                                scalar2=None, op0=Alu.mult,
                            )
                            nc.sync.dma_start(
                                out_d[m * P:(m + 1) * P, lo:lo + hf], dst)
                        else:
                            nc.scalar.activation(dst, src, Act.Copy,
                                                 scale=sca[:])
                            nc.scalar.dma_start(
                                out_d[m * P:(m + 1) * P, lo:lo + hf], dst)
    nc.compile()
    return nc


def _get_nc():
    if "nc" not in _CACHE:
        _CACHE["nc"] = _build_bass()
    return _CACHE["nc"]


def _make_in_maps(A_f32):
    import ml_dtypes

    f8 = ml_dtypes.float8_e4m3
    A8 = A_f32.astype(f8)
    # a_il[t, p, i, c] = A[t*256 + i*128 + p, c]
    a_il = np.ascontiguousarray(
        A8.reshape(KP, 2, P, N).transpose(0, 2, 1, 3)
    )
    AT8 = A8.T
    in_maps = []
    for c in range(NCORES):
        sl = slice(c * R, (c + 1) * R)
        # art_il[t, p, i, r] = A[row0 + r, t*256 + i*128 + p]
        art_il = np.ascontiguousarray(
            AT8[:, sl].reshape(KP, 2, P, R).transpose(0, 2, 1, 3)
        )
        in_maps.append({"a": a_il, "art": art_il})
    return in_maps


def kernel(A, w1a=None, w1b=None, w2a=None, **_unused):
    # w1a/w1b/w2a only enter the reference through a softmax over a
    # singleton axis (== 1.0), so the output does not depend on them.
    from concourse.bass_utils import run_bass_kernel_spmd

    A = np.asarray(A, dtype=np.float32)
    assert A.shape == (N, N), A.shape
    nc = _get_nc()
    in_maps = _make_in_maps(A)
    res = run_bass_kernel_spmd(nc, in_maps, core_ids=list(range(NCORES)))
    out = np.concatenate(
        [res.results[c]["out"] for c in range(NCORES)], axis=0
    )
    return out[None].astype(np.float32)
